# revision 44
# baseline (speedup 1.0000x reference)
"""Trainium2 Bass kernel for nn_MultiHeadAttention_5334349382389 (v3).

Sharding: 8 cores = 4 batches x 2 head-groups (4 heads each).
Core c handles batch b = c // 2, head-group g = c % 2 (heads 4g..4g+3).

Per-core math (fp16 matmuls, fp32 PSUM accumulate):
  qhT = (Wq_g/8) @ x_b^T + bq_g/8        [256, 1024]   (score scale folded into Wq)
  khT = Wk_g @ x_b^T + bk_g              [256, 1024]
  vh  = x_b @ Wv_g^T                     [1024, 256]   (bv folded into host-side bias)
  per head h: scoresT[k,q]; h==0 accumulates I @ edgeT on the PE into the
      score PSUM (edgeT is zeros on non-edge cores; Wq/bq head-0 slice
      zeroed on edge cores, so edge cores get scoresT == edgeT exactly)
  expT = exp(scoresT)                    (no max-subtraction; inputs bounded)
  outT_raw[d,q] accum over k-tiles with lhsT = [vh | ones] -> row 64 = denom
  OT = outT_raw[:64] * bcast(1/denom)
  partial = OT^T-contraction @ WoT_g     [1024, 512]
Host: out[b] = partial(b,0) + partial(b,1) + (bo + Wo @ bv).

v3 schedule (vs the 92.5us v2):
- dma_start only holds its engine ~0.7us (transfer is async); HBM ~360GB/s
  aggregate is the real limit, so DMA priority = per-queue FIFO order.
  Critical stream (twq/twk + xq both stripes + xk stripe 0) is split across
  sync/gpsimd/scalar and lands first; xv -> xk s1 -> edge -> wo trail.
- xq/xk packed stripe-major [P, NS, CD, STR] so projections chase the DMA
  per cd-chunk; first exp targeted ~12-13us (vs 27.4us).
- One flat software-pipelined loop over 32 (head, kt) steps, head order
  1,0,2,3. exp(step) emitted right after its score mms; PV(step) emitted
  two steps later (one step for kt7) so the PE never waits on the Act
  engine; PSUM: score ring bufs=2 (2 banks each), pv ring bufs=2, proj
  ring bufs=2 -> exactly 8 banks.
- PE filler work (v-projection pairs chasing xv, q-ch1/k-ch1 projections)
  is threaded into head-1 steps; deferred DVE hooks (biases, previous
  head's normalize) ride each step as in v2.
- Tail: no junk bridge; h3 normalize per stripe overlaps the output
  projection; oal casts rotate across DVE/gpsimd/scalar; stores rotate
  across the sync/gpsimd/scalar queues per m-tile.
"""

import os
import sys

sys.path.insert(0, "/opt/trn_rl_repo")

import numpy as np

B, SEQ, DIN, DO = 4, 1024, 512, 512
NH_ALL, DK = 8, 64
NHC = 4            # heads per core
DH = NHC * DK      # 256 per-core projected dims
P = 128
CD = DIN // P      # 4 contraction chunks for projections
CH = DH // P       # 2 dh chunks
KT = SEQ // P      # 8 k-tiles
STR = 512          # q-stripe (matmul free dim)
NS = SEQ // STR    # 2 stripes
TVW = NHC * (DK + 1) + DK - 1  # 323: per-k-tile aux width (4x65 + 63 pad)

NJUNK0 = int(os.environ.get("KERNEL_NJUNK0", "14"))

COMPUTE = os.environ.get("KERNEL_COMPUTE_DT", "fp16")  # fp16 | bf16 | fp32r

_nc = None


def _np_dt():
    import ml_dtypes

    return {
        "fp16": np.float16,
        "bf16": ml_dtypes.bfloat16,
        "fp32r": np.float32,
    }[COMPUTE]


def _build():
    global _nc
    if _nc is not None:
        return _nc
    import concourse.bacc as bacc
    import concourse.bass as bass
    import concourse.mybir as mybir
    import concourse.tile as tile

    f32 = mybir.dt.float32
    f32r = mybir.dt.float32r
    cdt = {
        "fp16": mybir.dt.float16,
        "bf16": mybir.dt.bfloat16,
        "fp32r": f32r,
    }[COMPUTE]
    Exp = mybir.ActivationFunctionType.Exp

    nc = bacc.Bacc("TRN2", target_bir_lowering=False, debug=False)

    eye_d = nc.dram_tensor("eye", (P, P), cdt, kind="ExternalInput")
    wq_d = nc.dram_tensor("wq", (P, CD * DH), cdt, kind="ExternalInput")
    wk_d = nc.dram_tensor("wk", (P, CD * DH), cdt, kind="ExternalInput")
    wv_d = nc.dram_tensor("wv", (P, CD * DH), cdt, kind="ExternalInput")
    wo_d = nc.dram_tensor("wo", (P, CH * DO), cdt, kind="ExternalInput")
    # xq/xk stripe-major: [P, NS, CD, STR]; xv k-tile-major: [P, KT, CD, P]
    xq_d = nc.dram_tensor("xq", (P, NS * CD * STR), cdt, kind="ExternalInput")
    xk_d = nc.dram_tensor("xk", (P, NS * CD * STR), cdt, kind="ExternalInput")
    xv_d = nc.dram_tensor("xv", (P, KT * CD * P), cdt, kind="ExternalInput")
    bqk = nc.dram_tensor("bqk", (2 * DH, 1), f32, kind="ExternalInput")
    edge = nc.dram_tensor("edge", (SEQ, SEQ), cdt, kind="ExternalInput")
    outp = nc.dram_tensor("outp", (SEQ, DO), cdt, kind="ExternalOutput")

    xq_r = xq_d.rearrange("p (s c n) -> p s c n", s=NS, c=CD)
    xk_r = xk_d.rearrange("p (s c n) -> p s c n", s=NS, c=CD)
    xv_r4 = xv_d.rearrange("p (t c j) -> p t c j", c=CD, j=P)
    edge_r2 = edge.rearrange("(t x p) n -> t p x n", x=2, p=P)
    out_r = outp.rearrange("(t p) n -> p t n", p=P)

    def sl(s):
        return slice(s * STR, (s + 1) * STR)

    with tile.TileContext(nc) as tc:
        with (
            tc.tile_pool(name="inp", bufs=1) as inp,
            tc.tile_pool(name="wts", bufs=1) as wts,
            tc.tile_pool(name="qkp", bufs=1) as qkp,
            tc.tile_pool(name="vhap", bufs=1) as vhap,
            tc.tile_pool(name="expp", bufs=8) as expp,
            tc.tile_pool(name="otp", bufs=1) as otp,
            tc.tile_pool(name="rrp", bufs=4) as rrp,
            tc.tile_pool(name="rbp", bufs=4) as rbp,
            tc.tile_pool(name="oalp", bufs=3) as oalp,
            tc.tile_pool(name="edgp", bufs=8) as edgp,
            # PSUM: 2*[P,SEQ] (4 banks) + 2*[P,STR] + 2*[P,STR] = 8 banks
            tc.tile_pool(name="bigp", bufs=2, space=bass.MemorySpace.PSUM) as bigp,
            tc.tile_pool(name="pvp", bufs=2, space=bass.MemorySpace.PSUM) as pvp,
            tc.tile_pool(name="prjp", bufs=2, space=bass.MemorySpace.PSUM) as prjp,
        ):
            # ---------------- tiles ----------------
            tjk = wts.tile([P, STR], cdt, tag="tjk")
            twq = wts.tile([P, CD, DH], cdt, tag="twq")
            twk = wts.tile([P, CD, DH], cdt, tag="twk")
            twv = wts.tile([P, CD, DH], cdt, tag="twv")
            two = wts.tile([P, CH, DO], cdt, tag="two")
            tb4 = wts.tile([P, 4, 1], f32, tag="tb4")
            teye = wts.tile([P, P], cdt, tag="teye")
            txq = inp.tile([P, NS, CD, STR], cdt, tag="txq")
            txk = inp.tile([P, NS, CD, STR], cdt, tag="txk")
            txv = inp.tile([P, KT, CD, P], cdt, tag="txv")
            tqh = qkp.tile([P, CH, SEQ], cdt, tag="tqh")
            khp = qkp.tile([P, NHC, SEQ], cdt, tag="khp")
            tvha = vhap.tile([P, KT, TVW], cdt, tag="tvha")
            tot = otp.tile([P, CH, SEQ], cdt, tag="tot")
            ed_pairs = [
                edgp.tile([P, 2, SEQ], cdt, tag="edg", name=f"edp{i}")
                for i in range(KT // 2)
            ]

            # ------- memsets: tjk on gpsimd (first op, gates junk); the rest
            # on DVE so the queues can start issuing DMAs immediately -------
            nc.gpsimd.memset(tjk, 0.0)
            # zero the unused partition-halves of khp (even heads: parts
            # 64-127, odd heads: parts 0-63) so score matmuls see zero weights
            nc.vector.memset(khp[0:DK, 1::2, :], 0.0)
            nc.vector.memset(khp[DK:P, 0::2, :], 0.0)
            # vh-aug tail pad + per-head ones columns (denominator rows)
            nc.vector.memset(tvha[:, :, NHC * (DK + 1) : TVW], 0.0)
            nc.vector.memset(
                tvha[:, :, 0 : NHC * (DK + 1)].rearrange(
                    "p t (h w) -> p t h w", w=DK + 1
                )[:, :, :, DK : DK + 1],
                1.0,
            )

            # ------- input DMAs.  Issue cost on the engine is ~0.7us; the
            # transfers drain asynchronously, so per-queue FIFO order is the
            # priority order.  sync (HWDGE) starts earliest, scalar (HWDGE)
            # next; the gpsimd SWDGE queue starts ~6us later, so it carries
            # only second/third-tier data.  scalar must be idle by the first
            # exp (~16us). -------
            # critical stream split evenly across sync+gpsimd (observed
            # ~130-200 GB/s each; scalar's queue is slow, ~60-100, and must
            # be free for exps, so it carries only small early items)
            # Measured queue rates under contention: gpsimd (SWDGE) ~200-240
            # KB/us, sync ~110, scalar bursty ~80-170 and must be free for
            # the exps.  gpsimd carries the bigger critical share.
            nc.sync.dma_start(
                out=twk, in_=wk_d.rearrange("p (c d) -> p c d", d=DH)
            )
            nc.gpsimd.dma_start(
                out=twq, in_=wq_d.rearrange("p (c d) -> p c d", d=DH)
            )
            nc.scalar.dma_start(out=tb4, in_=bqk.rearrange("(c p) o -> p c o", p=P))
            nc.sync.dma_start(out=txq[:, 0, 0:2], in_=xq_r[:, 0, 0:2])
            nc.gpsimd.dma_start(out=txq[:, 0, 2:4], in_=xq_r[:, 0, 2:4])
            nc.scalar.dma_start(out=txk[:, 0, 0:2], in_=xk_r[:, 0, 0:2])
            nc.sync.dma_start(out=txk[:, 0, 2:4], in_=xk_r[:, 0, 2:4])
            # second tier: k stripe 1 (phase A k-tiles 4-7), then v stream
            # (vproj fillers from pair-step A2), then q stripe 1 (phase B)
            nc.sync.dma_start(out=txk[:, 1, 0:2], in_=xk_r[:, 1, 0:2])
            nc.gpsimd.dma_start(out=txk[:, 1, 2:4], in_=xk_r[:, 1, 2:4])
            nc.gpsimd.dma_start(
                out=twv, in_=wv_d.rearrange("p (c d) -> p c d", d=DH)
            )
            nc.gpsimd.dma_start(out=txv[:, 0:2], in_=xv_r4[:, 0:2])
            nc.sync.dma_start(out=txq[:, 1, 0:2], in_=xq_r[:, 1, 0:2])
            nc.gpsimd.dma_start(out=txq[:, 1, 2:4], in_=xq_r[:, 1, 2:4])
            nc.scalar.dma_start(out=teye, in_=eye_d[:, :])
            nc.gpsimd.dma_start(out=txv[:, 2:4], in_=xv_r4[:, 2:4])
            nc.sync.dma_start(out=txv[:, 4:6], in_=xv_r4[:, 4:6])
            nc.gpsimd.dma_start(out=txv[:, 6:8], in_=xv_r4[:, 6:8])
            # third tier: edge + wo (needed from head 0 / the output stage)
            nc.sync.dma_start(out=ed_pairs[0], in_=edge_r2[0])
            nc.sync.dma_start(out=ed_pairs[1], in_=edge_r2[1])
            nc.gpsimd.dma_start(out=ed_pairs[2], in_=edge_r2[2])
            nc.sync.dma_start(out=ed_pairs[3], in_=edge_r2[3])
            nc.gpsimd.dma_start(
                out=two, in_=wo_d.rearrange("p (c d) -> p c d", d=DO)
            )

            # PE clock-ramp filler on the memset tile (no DMA dependency)
            def junk(n, pool=None, name="jt"):
                jt = (pool or prjp).tile([P, STR], f32, tag="prj" if pool is None else "big", name=name)
                for _ in range(n):
                    nc.tensor.matmul(
                        jt[:], lhsT=tjk[:, 0:P], rhs=tjk[:], start=True, stop=True
                    )

            junk(NJUNK0)

            # ------- critical projections: q-ch0 s0 first (its xq chunks
            # land first), then k-ch0 s0; k-ch0 s1 (kcB) is deferred via
            # wait-until so its txk-s1 DMA wait can't block the first
            # scores in the in-order PE queue. -------
            ptqa = bigp.tile([P, STR], f32, tag="big")
            for cd in range(CD):
                nc.tensor.matmul(
                    ptqa[:],
                    lhsT=twq[:, cd, 0:P],
                    rhs=txq[:, 0, cd, :],
                    start=(cd == 0),
                    stop=(cd == CD - 1),
                )
            kcA = pvp.tile([P, STR], f32, tag="pv")
            for cd in range(CD):
                nc.tensor.matmul(
                    kcA[:],
                    lhsT=twk[:, cd, 0:P],
                    rhs=txk[:, 0, cd, :],
                    start=(cd == 0),
                    stop=(cd == CD - 1),
                )
            # biases for the first scores as soon as their psum lands
            nc.vector.tensor_scalar_add(
                out=tqh[:, 0, sl(0)], in0=ptqa[:], scalar1=tb4[:, 0, :]
            )
            nc.vector.tensor_scalar_add(
                out=khp[0:DK, 0, sl(0)], in0=kcA[0:DK, :], scalar1=tb4[0:DK, 2, :]
            )
            nc.vector.tensor_scalar_add(
                out=khp[DK:P, 1, sl(0)], in0=kcA[DK:P, :], scalar1=tb4[DK:P, 2, :]
            )
            kcB = pvp.tile([P, STR], f32, tag="pv")
            with tc.tile_wait_until(0.0166):
                for cd in range(CD):
                    nc.tensor.matmul(
                        kcB[:],
                        lhsT=twk[:, cd, 0:P],
                        rhs=txk[:, 1, cd, :],
                        start=(cd == 0),
                        stop=(cd == CD - 1),
                    )
                nc.vector.tensor_scalar_add(
                    out=khp[0:DK, 0, sl(1)], in0=kcB[0:DK, :], scalar1=tb4[0:DK, 2, :]
                )
                nc.vector.tensor_scalar_add(
                    out=khp[DK:P, 1, sl(1)], in0=kcB[DK:P, :], scalar1=tb4[DK:P, 2, :]
                )

            # ---------------- flat software-pipelined main loop -----------
            # steps: (head, kt) in head order 1, 0, 2, 3.
            HEADS = (1, 0, 2, 3)
            steps = [(h, kt) for h in HEADS for kt in range(KT)]

            # PE fillers threaded into head-1 steps (index within head 1).
            # vproj pair j covers k-tiles 2j, 2j+1 (8 mms each); ch1
            # projections (ptqb = q-ch1, ptk2 = k-ch1) ride the prjp ring.
            def vproj_pair(j):
                vp = prjp.tile([P, STR], f32, tag="prj")
                for u in range(2):
                    for cd in range(CD):
                        nc.tensor.matmul(
                            vp[:, u * DH : (u + 1) * DH],
                            lhsT=txv[:, 2 * j + u, cd, :],
                            rhs=twv[:, cd, :],
                            start=(cd == 0),
                            stop=(cd == CD - 1),
                        )
                # copy into the augmented-vh layout (DVE)
                nc.vector.tensor_copy(
                    out=tvha[:, 2 * j : 2 * j + 2, 0 : NHC * (DK + 1)].rearrange(
                        "p t (h w) -> p t h w", w=DK + 1
                    )[:, :, :, 0:DK],
                    in_=vp[:].rearrange("p (t h d) -> p t h d", t=2, h=NHC),
                )

            def qch1_stripe(s):
                pt = prjp.tile([P, STR], f32, tag="prj")
                for cd in range(CD):
                    nc.tensor.matmul(
                        pt[:],
                        lhsT=twq[:, cd, P : 2 * P],
                        rhs=txq[:, s, cd, :],
                        start=(cd == 0),
                        stop=(cd == CD - 1),
                    )
                nc.vector.tensor_scalar_add(
                    out=tqh[:, 1, sl(s)], in0=pt[:], scalar1=tb4[:, 1, :]
                )

            def kch1_stripe(s):
                pt = prjp.tile([P, STR], f32, tag="prj")
                for cd in range(CD):
                    nc.tensor.matmul(
                        pt[:],
                        lhsT=twk[:, cd, P : 2 * P],
                        rhs=txk[:, s, cd, :],
                        start=(cd == 0),
                        stop=(cd == CD - 1),
                    )
                nc.vector.tensor_scalar_add(
                    out=khp[0:DK, 2, sl(s)], in0=pt[0:DK, :], scalar1=tb4[0:DK, 3, :]
                )
                nc.vector.tensor_scalar_add(
                    out=khp[DK:P, 3, sl(s)], in0=pt[DK:P, :], scalar1=tb4[DK:P, 3, :]
                )

            # pre-accumulated ch0 output-projection for m-tiles 0/1 (their
            # prjp slots stay pinned until the tail adds ch1 on top)
            po_pre = {}

            def po_ch0(m):
                po = prjp.tile([P, DO], f32, tag="prj", name=f"pre{m}")
                nc.tensor.matmul(
                    po[:],
                    lhsT=tot[:, 0, m * P : (m + 1) * P],
                    rhs=two[:, 0, :],
                    start=True,
                    stop=False,
                    skip_group_check=True,
                )
                po_pre[m] = po

            def qch0_s1():
                pt = prjp.tile([P, STR], f32, tag="prj")
                for cd in range(CD):
                    nc.tensor.matmul(
                        pt[:],
                        lhsT=twq[:, cd, 0:P],
                        rhs=txq[:, 1, cd, :],
                        start=(cd == 0),
                        stop=(cd == CD - 1),
                    )
                nc.vector.tensor_scalar_add(
                    out=tqh[:, 0, sl(1)], in0=pt[:], scalar1=tb4[:, 0, :]
                )

            fillers = {
                (0, 0): lambda: kch1_stripe(1),
                (3, 3): lambda: po_ch0(0),
                (3, 4): lambda: po_ch0(1),
            }
            fillers_A = {
                2: lambda: vproj_pair(0),
                3: lambda: (qch0_s1(), vproj_pair(1)),
            }
            fillers_B = {
                0: lambda: vproj_pair(2),
                1: lambda: vproj_pair(3),
                2: lambda: qch1_stripe(0),
                3: lambda: (qch1_stripe(1), kch1_stripe(0)),
            }

            # deferred normalize for the previous head, hooked into the next
            # head's step 1 (PV of kt7 lands there too); the reciprocal
            # reads the denominator row straight out of PSUM
            def norm_stripe(h, pvs, s):
                rr = rrp.tile([1, STR], f32, tag="rr")
                rs = rrp.tile([1, STR], f32, tag="rs")
                nc.vector.tensor_copy(out=rs[:], in_=pvs[s][DK : DK + 1, :])
                nc.vector.reciprocal_approx_fast(out=rr[:], in_=rs[:])
                rb = rbp.tile([DK, STR], f32, tag="rb")
                nc.gpsimd.partition_broadcast(rb[:], rr[:])
                ch, off = h // 2, (h % 2) * DK
                nc.vector.tensor_mul(
                    tot[off : off + DK, ch, sl(s)], pvs[s][0:DK, :], rb[:]
                )

            # main loop state
            pv_by_head = {}
            te_by_step = {}
            te_h1 = {}
            prev_head = {1: None, 0: 1, 2: 0, 3: 2}

            def emit_pv(h, kt, stop):
                pvs = pv_by_head[h]
                te = te_by_step[(h, kt)]
                for s in range(NS):
                    nc.tensor.matmul(
                        pvs[s][:],
                        lhsT=tvha[:, kt, h * (DK + 1) : h * (DK + 1) + P],
                        rhs=te[:, sl(s)],
                        start=(kt == 0),
                        stop=stop,
                    )

            def pv1_mm(kt, s, stop):
                te, u = te_h1[(kt, s)]
                nc.tensor.matmul(
                    pv_by_head[1][s][:],
                    lhsT=tvha[:, kt, 1 * (DK + 1) : 1 * (DK + 1) + P],
                    rhs=te[:, sl(u)],
                    start=(kt == 0),
                    stop=stop,
                )

            def pv1_pair(p, s, stop_last):
                pv1_mm(2 * p, s, False)
                pv1_mm(2 * p + 1, s, stop_last)

            # Logical clock: wait-until hints make the Tile scheduler's sim
            # order instructions the way the real hardware needs them —
            # scores/exp first within a step, then fillers/hooks, then the
            # lagged PVs.  (The sim's fast DMA model otherwise hoists filler
            # matmuls ahead of critical scores; the in-order PE queue then
            # stalls on late DMAs.)
            #
            # Head 1 runs as 8 pair-steps: stripe 0 of k-tile pairs 0..3
            # (phase A, needs only xq stripe 0), then stripe 1 (phase B) —
            # xq stripe 1 thus leaves the critical DMA path.  Each pair-step
            # exps one [P, SEQ] tile holding two k-tiles' half-scores, so
            # the Act engine efficiency is unchanged.
            TA, TSTEP = 15.0, 1.2

            pv_by_head[1] = (
                pvp.tile([P, STR], f32, tag="pv", name="pv1s0"),
                pvp.tile([P, STR], f32, tag="pv", name="pv1s1"),
            )
            for p in range(8):
                s, j = p // 4, p % 4
                base = TA + TSTEP * p
                with tc.tile_wait_until(base / 1000.0):
                    stt = bigp.tile([P, SEQ], f32, tag="big")
                    for u in (0, 1):
                        kt = 2 * j + u
                        nc.tensor.matmul(
                            stt[:, sl(u)],
                            lhsT=khp[:, 1, kt * P : (kt + 1) * P],
                            rhs=tqh[:, 0, sl(s)],
                            start=True,
                            stop=True,
                        )
                    te = expp.tile([P, SEQ], cdt, tag="expT")
                    nc.scalar.activation(out=te, in_=stt[:], func=Exp)
                    for u in (0, 1):
                        te_h1[(2 * j + u, s)] = (te, u)
                f = (fillers_A if s == 0 else fillers_B).get(j)
                if f is not None:
                    with tc.tile_wait_until((base + 0.4) / 1000.0):
                        f()
                if p >= 3:
                    # lag-3 over the pair-step sequence (vproj pairs chase
                    # the xv DMA stream one step ahead of their PVs)
                    pp = p - 3
                    ps, pj = pp // 4, pp % 4
                    with tc.tile_wait_until((base + 0.8) / 1000.0):
                        pv1_pair(pj, ps, stop_last=(pj == 3))

            # heads 0, 2, 3 as full steps
            T0 = TA + TSTEP * 8
            steps = [(h, kt) for h in (0, 2, 3) for kt in range(KT)]
            for i, (h, kt) in enumerate(steps):
                ch = h // 2
                base = T0 + TSTEP * i
                # allocate this head's pv tiles at its first step
                if kt == 0:
                    pv_by_head[h] = (
                        pvp.tile([P, STR], f32, tag="pv", name=f"pv{h}s0"),
                        pvp.tile([P, STR], f32, tag="pv", name=f"pv{h}s1"),
                    )
                with tc.tile_wait_until(base / 1000.0):
                    # scores for (h, kt); head 0 accumulates I @ edgeT on top
                    stt = bigp.tile([P, SEQ], f32, tag="big")
                    for s in range(NS):
                        nc.tensor.matmul(
                            stt[:, sl(s)],
                            lhsT=khp[:, h, kt * P : (kt + 1) * P],
                            rhs=tqh[:, ch, sl(s)],
                            start=True,
                            stop=(h != 0),
                        )
                        if h == 0:
                            nc.tensor.matmul(
                                stt[:, sl(s)],
                                lhsT=teye[:],
                                rhs=ed_pairs[kt // 2][:, kt % 2, sl(s)],
                                start=False,
                                stop=True,
                            )
                    # exp on the Act engine
                    te = expp.tile([P, SEQ], cdt, tag="expT")
                    nc.scalar.activation(out=te, in_=stt[:], func=Exp)
                    te_by_step[(h, kt)] = te
                # PE fillers for this step
                f = fillers.get((h, kt))
                if f is not None:
                    with tc.tile_wait_until((base + 0.4) / 1000.0):
                        f()
                # lagged PV matmuls; head 1's stripe-1 tail PVs land in
                # head 0's first two steps, followed by the normalize hooks
                # that free the pvp ring
                ph = prev_head[h]
                if h == 0 and kt == 0:
                    with tc.tile_wait_until((base + 0.8) / 1000.0):
                        pv1_pair(1, 1, stop_last=False)
                elif h == 0 and kt == 1:
                    with tc.tile_wait_until((base + 0.5) / 1000.0):
                        pv1_pair(2, 1, stop_last=False)
                        pv1_pair(3, 1, stop_last=True)
                        norm_stripe(1, pv_by_head[1], 0)
                        norm_stripe(1, pv_by_head[1], 1)
                elif kt == 0 and ph is not None:
                    with tc.tile_wait_until((base + 0.8) / 1000.0):
                        emit_pv(ph, KT - 2, stop=False)
                elif kt == 1 and ph is not None:
                    with tc.tile_wait_until((base + 0.5) / 1000.0):
                        emit_pv(ph, KT - 1, stop=True)
                        norm_stripe(ph, pv_by_head[ph], 0)
                        norm_stripe(ph, pv_by_head[ph], 1)
                elif kt >= 2:
                    with tc.tile_wait_until((base + 0.8) / 1000.0):
                        emit_pv(h, kt - 2, stop=False)

            # ---------------- tail ----------------
            TT = T0 + TSTEP * len(steps)
            h_last = HEADS[-1]
            with tc.tile_wait_until(TT / 1000.0):
                emit_pv(h_last, KT - 2, stop=False)
                junk(2, pool=bigp, name="jtt0")
                emit_pv(h_last, KT - 1, stop=True)

            # h3 normalize, per stripe; stripe 0 gates out-proj m 0-3.
            # denominator copy on the Act engine (idle after the last exp)
            pvs3 = pv_by_head[h_last]
            ch3, off3 = h_last // 2, (h_last % 2) * DK

            def norm_tail(s):
                rs = rrp.tile([1, STR], f32, tag="rs")
                if s == 0:
                    nc.scalar.copy(out=rs[:], in_=pvs3[s][DK : DK + 1, :])
                else:
                    nc.vector.tensor_copy(out=rs[:], in_=pvs3[s][DK : DK + 1, :])
                rr = rrp.tile([1, STR], f32, tag="rr")
                nc.vector.reciprocal_approx_fast(out=rr[:], in_=rs[:])
                rb = rbp.tile([DK, STR], f32, tag="rb")
                nc.gpsimd.partition_broadcast(rb[:], rr[:])
                nc.vector.tensor_mul(
                    tot[off3 : off3 + DK, ch3, sl(s)], pvs3[s][0:DK, :], rb[:]
                )

            with tc.tile_wait_until((TT + 0.6) / 1000.0):
                norm_tail(0)
                junk(9, pool=bigp, name="jtt1")
                norm_tail(1)

            # output projection, one po mm per chunk per m-tile, spread over
            # six 1-bank psum slots (2 pinned pre-tiles + pvp + bigp ring);
            # casts alternate DVE/Act, stores alternate the sync/gpsimd
            # queues (the scalar engine stays cast-only)
            cast_ops = [
                lambda o, i: nc.vector.tensor_copy(out=o, in_=i),
                lambda o, i: nc.scalar.copy(out=o, in_=i),
            ]
            store_eng = [nc.sync, nc.gpsimd]

            def po_tail(m, po, ch0_done):
                if not ch0_done:
                    nc.tensor.matmul(
                        po[:],
                        lhsT=tot[:, 0, m * P : (m + 1) * P],
                        rhs=two[:, 0, :],
                        start=True,
                        stop=False,
                        skip_group_check=True,
                    )
                nc.tensor.matmul(
                    po[:],
                    lhsT=tot[:, 1, m * P : (m + 1) * P],
                    rhs=two[:, 1, :],
                    start=False,
                    stop=True,
                    skip_group_check=True,
                )
                oal = oalp.tile([P, DO], cdt, tag="oall")
                cast_ops[m % 2](oal[:], po[:])
                store_eng[m % 2].dma_start(out=out_r[:, m], in_=oal[:])

            with tc.tile_wait_until((TT + 1.4) / 1000.0):
                po_tail(0, po_pre[0], True)
                po_tail(1, po_pre[1], True)
            with tc.tile_wait_until((TT + 2.0) / 1000.0):
                for m in (2, 3):
                    po = pvp.tile([P, DO], f32, tag="pv", name=f"po{m}")
                    po_tail(m, po, False)
            with tc.tile_wait_until((TT + 2.6) / 1000.0):
                for m in (4, 5):
                    po = bigp.tile([P, DO], f32, tag="big", name=f"po{m}")
                    po_tail(m, po, False)
            with tc.tile_wait_until((TT + 3.2) / 1000.0):
                for m in (6, 7):
                    po = prjp.tile([P, DO], f32, tag="prj", name=f"po{m}")
                    po_tail(m, po, False)

    nc.compile()
    _nc = nc
    return nc


def _in_maps(q, k, v, edge_matrix, Wq, bq, Wk, bk, Wv, Wo):
    dt = _np_dt()
    zeros_edge = np.zeros((SEQ, SEQ), dt)
    edge_t = np.ascontiguousarray(edge_matrix.T).astype(dt)

    def re_cp(m):
        # [C*P, D] -> [P, C*D] (partition-major packing of "(c p) d -> p c d")
        cp, d = m.shape
        return np.ascontiguousarray(
            m.reshape(cp // P, P, d).transpose(1, 0, 2).reshape(P, -1)
        )

    def re_sp(m):
        # [CD*P, NS*STR] -> [P, NS*CD*STR] stripe-major packing
        return np.ascontiguousarray(
            m.reshape(CD, P, NS, STR).transpose(1, 2, 0, 3).reshape(P, -1)
        )

    def re_st(m):
        # [CD*P, KT*P] -> [P, KT*CD*P]: kt-major packing for the v stream
        return np.ascontiguousarray(
            m.reshape(CD, P, KT, P).transpose(1, 2, 0, 3).reshape(P, -1)
        )

    xt = {}
    for b in range(B):
        xt[b] = (
            re_sp(np.ascontiguousarray(q[b].T).astype(dt)),
            re_sp(np.ascontiguousarray(k[b].T).astype(dt)),
            re_st(np.ascontiguousarray(v[b].T).astype(dt)),
        )
    maps = []
    for c in range(8):
        b, g = c // 2, c % 2
        is_edge = g == 0 and b < 2
        rows = slice(g * DH, (g + 1) * DH)
        wq_c = np.ascontiguousarray(Wq[rows].T) * np.float32(1.0 / 8.0)
        bq_c = (bq[rows] * np.float32(1.0 / 8.0)).copy()
        if is_edge:
            wq_c[:, 0:DK] = 0.0
            bq_c[0:DK] = 0.0
        maps.append(
            {
                "eye": np.eye(P, dtype=dt),
                "wq": re_cp(wq_c.astype(dt)),
                "wk": re_cp(np.ascontiguousarray(Wk[rows].T).astype(dt)),
                "wv": re_cp(np.ascontiguousarray(Wv[rows].T).astype(dt)),
                "wo": re_cp(np.ascontiguousarray(Wo[:, rows].T).astype(dt)),
                "xq": xt[b][0],
                "xk": xt[b][1],
                "xv": xt[b][2],
                "bqk": np.concatenate([bq_c, bk[rows]]).reshape(2 * DH, 1),
                "edge": edge_t if is_edge else zeros_edge,
            }
        )
    return maps


def _ensure_ntff_hook():
    """Register the axon NTFF profile hook if the image's antenv lacks it."""
    import contextlib
    import ctypes
    import types

    try:
        from antenv.axon_hooks import get_axon_ntff_profile_hook  # noqa: F401
        return
    except ImportError:
        pass

    so_path = "/opt/axon/libaxon_pjrt.so"
    try:
        lib = ctypes.CDLL(so_path)
    except OSError:
        return
    if not hasattr(lib, "axon_start_nrt_profile"):
        return
    lib.axon_start_nrt_profile.argtypes = [
        ctypes.POINTER(ctypes.c_int64),
        ctypes.c_size_t,
    ]
    lib.axon_start_nrt_profile.restype = ctypes.c_int64
    lib.axon_stop_nrt_profile.argtypes = [ctypes.c_char_p]
    lib.axon_stop_nrt_profile.restype = ctypes.c_int64

    @contextlib.contextmanager
    def _hook(output_dir, device_ids):
        import jax

        jax.devices()
        if device_ids:
            ids = (ctypes.c_int64 * len(device_ids))(*device_ids)
            rc = lib.axon_start_nrt_profile(ids, len(device_ids))
        else:
            rc = lib.axon_start_nrt_profile(None, 0)
        if rc != 0:
            raise RuntimeError(f"axon_start_nrt_profile rc={rc}")
        try:
            yield
        finally:
            n = lib.axon_stop_nrt_profile(str(output_dir).encode())
            if n < 0:
                raise RuntimeError(f"axon_stop_nrt_profile rc={n}")

    _state = {"hook": _hook}
    mod = types.ModuleType("antenv.axon_hooks")
    mod.get_axon_ntff_profile_hook = lambda: _state["hook"]
    mod.set_axon_ntff_profile_hook = lambda h: _state.__setitem__("hook", h)
    import antenv

    antenv.axon_hooks = mod
    sys.modules["antenv.axon_hooks"] = mod


def kernel(q, k, v, edge_matrix, Wq, bq, Wk, bk, Wv, bv, Wo, bo, _trace=False):
    from concourse.bass_utils import run_bass_kernel_spmd

    if _trace:
        _ensure_ntff_hook()

    q, k, v = (np.asarray(t, np.float32) for t in (q, k, v))
    edge_matrix = np.asarray(edge_matrix, np.float32)
    Wq, bq, Wk, bk, Wv, bv, Wo, bo = (
        np.asarray(t, np.float32) for t in (Wq, bq, Wk, bk, Wv, bv, Wo, bo)
    )

    nc = _build()
    maps = _in_maps(q, k, v, edge_matrix, Wq, bq, Wk, bk, Wv, Wo)
    res = run_bass_kernel_spmd(nc, maps, core_ids=list(range(8)), trace=_trace)

    bo_eff = bo + Wo @ bv
    out = np.empty((B, SEQ, DO), np.float32)
    for b in range(B):
        out[b] = res.results[2 * b]["outp"] + res.results[2 * b + 1]["outp"] + bo_eff
    if _trace:
        return out, res
    return out


# revision 46
# speedup vs baseline: 1.1597x; 1.1597x over previous
"""Trainium2 Bass kernel for nn_MultiHeadAttention_5334349382389 (v3).

Sharding: 8 cores = 4 batches x 2 head-groups (4 heads each).
Core c handles batch b = c // 2, head-group g = c % 2 (heads 4g..4g+3).

Per-core math (fp16 matmuls, fp32 PSUM accumulate):
  qhT = (Wq_g/8) @ x_b^T + bq_g/8        [256, 1024]   (score scale folded into Wq)
  khT = Wk_g @ x_b^T + bk_g              [256, 1024]
  vh  = x_b @ Wv_g^T                     [1024, 256]   (bv folded into host-side bias)
  per head h: scoresT[k,q]; h==0 accumulates I @ edgeT on the PE into the
      score PSUM (edgeT is zeros on non-edge cores; Wq/bq head-0 slice
      zeroed on edge cores, so edge cores get scoresT == edgeT exactly)
  expT = exp(scoresT)                    (no max-subtraction; inputs bounded)
  outT_raw[d,q] accum over k-tiles with lhsT = [vh | ones] -> row 64 = denom
  OT = outT_raw[:64] * bcast(1/denom)
  partial = OT^T-contraction @ WoT_g     [1024, 512]
Host: out[b] = partial(b,0) + partial(b,1) + (bo + Wo @ bv).

v3 schedule (vs the 92.5us v2):
- dma_start only holds its engine ~0.7us (transfer is async); HBM ~360GB/s
  aggregate is the real limit, so DMA priority = per-queue FIFO order.
  Critical stream (twq/twk + xq both stripes + xk stripe 0) is split across
  sync/gpsimd/scalar and lands first; xv -> xk s1 -> edge -> wo trail.
- xq/xk packed stripe-major [P, NS, CD, STR] so projections chase the DMA
  per cd-chunk; first exp targeted ~12-13us (vs 27.4us).
- One flat software-pipelined loop over 32 (head, kt) steps, head order
  1,0,2,3. exp(step) emitted right after its score mms; PV(step) emitted
  two steps later (one step for kt7) so the PE never waits on the Act
  engine; PSUM: score ring bufs=2 (2 banks each), pv ring bufs=2, proj
  ring bufs=2 -> exactly 8 banks.
- PE filler work (v-projection pairs chasing xv, q-ch1/k-ch1 projections)
  is threaded into head-1 steps; deferred DVE hooks (biases, previous
  head's normalize) ride each step as in v2.
- Tail: no junk bridge; h3 normalize per stripe overlaps the output
  projection; oal casts rotate across DVE/gpsimd/scalar; stores rotate
  across the sync/gpsimd/scalar queues per m-tile.
"""

import os
import sys

sys.path.insert(0, "/opt/trn_rl_repo")

import numpy as np

B, SEQ, DIN, DO = 4, 1024, 512, 512
NH_ALL, DK = 8, 64
NHC = 4            # heads per core
DH = NHC * DK      # 256 per-core projected dims
P = 128
CD = DIN // P      # 4 contraction chunks for projections
CH = DH // P       # 2 dh chunks
KT = SEQ // P      # 8 k-tiles
STR = 512          # q-stripe (matmul free dim)
NS = SEQ // STR    # 2 stripes
TVW = NHC * (DK + 1) + DK - 1  # 323: per-k-tile aux width (4x65 + 63 pad)

NJUNK0 = int(os.environ.get("KERNEL_NJUNK0", "14"))

COMPUTE = os.environ.get("KERNEL_COMPUTE_DT", "fp16")  # fp16 | bf16 | fp32r

_nc = None


def _np_dt():
    import ml_dtypes

    return {
        "fp16": np.float16,
        "bf16": ml_dtypes.bfloat16,
        "fp32r": np.float32,
    }[COMPUTE]


def _build():
    global _nc
    if _nc is not None:
        return _nc
    import concourse.bacc as bacc
    import concourse.bass as bass
    import concourse.mybir as mybir
    import concourse.tile as tile

    f32 = mybir.dt.float32
    f32r = mybir.dt.float32r
    cdt = {
        "fp16": mybir.dt.float16,
        "bf16": mybir.dt.bfloat16,
        "fp32r": f32r,
    }[COMPUTE]
    Exp = mybir.ActivationFunctionType.Exp

    nc = bacc.Bacc("TRN2", target_bir_lowering=False, debug=False)

    eye_d = nc.dram_tensor("eye", (P, P), cdt, kind="ExternalInput")
    wq_d = nc.dram_tensor("wq", (P, CD * DH), cdt, kind="ExternalInput")
    wk_d = nc.dram_tensor("wk", (P, CD * DH), cdt, kind="ExternalInput")
    wv_d = nc.dram_tensor("wv", (P, CD * DH), cdt, kind="ExternalInput")
    wo_d = nc.dram_tensor("wo", (P, CH * DO), cdt, kind="ExternalInput")
    # xq/xk stripe-major: [P, NS, CD, STR]; xv k-tile-major: [P, KT, CD, P]
    xq_d = nc.dram_tensor("xq", (P, NS * CD * STR), cdt, kind="ExternalInput")
    xk_d = nc.dram_tensor("xk", (P, NS * CD * STR), cdt, kind="ExternalInput")
    xv_d = nc.dram_tensor("xv", (P, KT * CD * P), cdt, kind="ExternalInput")
    bqk = nc.dram_tensor("bqk", (2 * DH, 1), f32, kind="ExternalInput")
    edge = nc.dram_tensor("edge", (SEQ, SEQ), cdt, kind="ExternalInput")
    outp = nc.dram_tensor("outp", (SEQ, DO), cdt, kind="ExternalOutput")

    xq_r = xq_d.rearrange("p (s c n) -> p s c n", s=NS, c=CD)
    xk_r = xk_d.rearrange("p (s c n) -> p s c n", s=NS, c=CD)
    xv_r4 = xv_d.rearrange("p (t c j) -> p t c j", c=CD, j=P)
    edge_r2 = edge.rearrange("(t x p) n -> t p x n", x=2, p=P)
    out_r = outp.rearrange("(t p) n -> p t n", p=P)

    def sl(s):
        return slice(s * STR, (s + 1) * STR)

    with tile.TileContext(nc) as tc:
        with (
            tc.tile_pool(name="inp", bufs=1) as inp,
            tc.tile_pool(name="wts", bufs=1) as wts,
            tc.tile_pool(name="qkp", bufs=1) as qkp,
            tc.tile_pool(name="vhap", bufs=1) as vhap,
            tc.tile_pool(name="expp", bufs=8) as expp,
            tc.tile_pool(name="otp", bufs=1) as otp,
            tc.tile_pool(name="rrp", bufs=4) as rrp,
            tc.tile_pool(name="rbp", bufs=4) as rbp,
            tc.tile_pool(name="oalp", bufs=3) as oalp,
            tc.tile_pool(name="edgp", bufs=8) as edgp,
            # PSUM: 2*[P,SEQ] (4 banks) + 2*[P,STR] + 2*[P,STR] = 8 banks
            tc.tile_pool(name="bigp", bufs=2, space=bass.MemorySpace.PSUM) as bigp,
            tc.tile_pool(name="pvp", bufs=2, space=bass.MemorySpace.PSUM) as pvp,
            tc.tile_pool(name="prjp", bufs=2, space=bass.MemorySpace.PSUM) as prjp,
        ):
            # ---------------- tiles ----------------
            tjk = wts.tile([P, STR], cdt, tag="tjk")
            twq = wts.tile([P, CD, DH], cdt, tag="twq")
            twk = wts.tile([P, CD, DH], cdt, tag="twk")
            twv = wts.tile([P, CD, DH], cdt, tag="twv")
            two = wts.tile([P, CH, DO], cdt, tag="two")
            tb4 = wts.tile([P, 4, 1], f32, tag="tb4")
            teye = wts.tile([P, P], cdt, tag="teye")
            txq = inp.tile([P, NS, CD, STR], cdt, tag="txq")
            txk = inp.tile([P, NS, CD, STR], cdt, tag="txk")
            txv = inp.tile([P, KT, CD, P], cdt, tag="txv")
            tqh = qkp.tile([P, CH, SEQ], cdt, tag="tqh")
            khp = qkp.tile([P, NHC, SEQ], cdt, tag="khp")
            tvha = vhap.tile([P, KT, TVW], cdt, tag="tvha")
            tot = otp.tile([P, CH, SEQ], cdt, tag="tot")
            ed_pairs = [
                edgp.tile([P, 2, SEQ], cdt, tag="edg", name=f"edp{i}")
                for i in range(KT // 2)
            ]

            # ------- memsets: tjk on gpsimd (first op, gates junk); the rest
            # on DVE so the queues can start issuing DMAs immediately -------
            nc.gpsimd.memset(tjk, 0.0)
            # zero the unused partition-halves of khp (even heads: parts
            # 64-127, odd heads: parts 0-63) so score matmuls see zero weights
            nc.vector.memset(khp[0:DK, 1::2, :], 0.0)
            nc.vector.memset(khp[DK:P, 0::2, :], 0.0)
            # vh-aug tail pad + per-head ones columns (denominator rows)
            nc.vector.memset(tvha[:, :, NHC * (DK + 1) : TVW], 0.0)
            nc.vector.memset(
                tvha[:, :, 0 : NHC * (DK + 1)].rearrange(
                    "p t (h w) -> p t h w", w=DK + 1
                )[:, :, :, DK : DK + 1],
                1.0,
            )

            # ------- input DMAs.  Issue cost on the engine is ~0.7us; the
            # transfers drain asynchronously, so per-queue FIFO order is the
            # priority order.  sync (HWDGE) starts earliest, scalar (HWDGE)
            # next; the gpsimd SWDGE queue starts ~6us later, so it carries
            # only second/third-tier data.  scalar must be idle by the first
            # exp (~16us). -------
            # critical stream split evenly across sync+gpsimd (observed
            # ~130-200 GB/s each; scalar's queue is slow, ~60-100, and must
            # be free for exps, so it carries only small early items)
            # Measured queue rates under contention: gpsimd (SWDGE) ~200-240
            # KB/us, sync ~110, scalar bursty ~80-170 and must be free for
            # the exps.  gpsimd carries the bigger critical share.
            nc.sync.dma_start(
                out=twk, in_=wk_d.rearrange("p (c d) -> p c d", d=DH)
            )
            nc.gpsimd.dma_start(
                out=twq, in_=wq_d.rearrange("p (c d) -> p c d", d=DH)
            )
            nc.scalar.dma_start(out=tb4, in_=bqk.rearrange("(c p) o -> p c o", p=P))
            nc.sync.dma_start(out=txq[:, 0, 0:2], in_=xq_r[:, 0, 0:2])
            nc.gpsimd.dma_start(out=txq[:, 0, 2:4], in_=xq_r[:, 0, 2:4])
            nc.scalar.dma_start(out=txk[:, 0, 0:2], in_=xk_r[:, 0, 0:2])
            nc.sync.dma_start(out=txk[:, 0, 2:4], in_=xk_r[:, 0, 2:4])
            # second tier: k stripe 1 (phase A k-tiles 4-7), then v stream
            # (vproj fillers from pair-step A2), then q stripe 1 (phase B)
            nc.sync.dma_start(out=txk[:, 1, 0:2], in_=xk_r[:, 1, 0:2])
            nc.gpsimd.dma_start(out=txk[:, 1, 2:4], in_=xk_r[:, 1, 2:4])
            nc.gpsimd.dma_start(
                out=twv, in_=wv_d.rearrange("p (c d) -> p c d", d=DH)
            )
            nc.gpsimd.dma_start(out=txv[:, 0:2], in_=xv_r4[:, 0:2])
            nc.sync.dma_start(out=txq[:, 1, 0:2], in_=xq_r[:, 1, 0:2])
            nc.gpsimd.dma_start(out=txq[:, 1, 2:4], in_=xq_r[:, 1, 2:4])
            nc.scalar.dma_start(out=teye, in_=eye_d[:, :])
            nc.gpsimd.dma_start(out=txv[:, 2:4], in_=xv_r4[:, 2:4])
            nc.sync.dma_start(out=txv[:, 4:6], in_=xv_r4[:, 4:6])
            nc.gpsimd.dma_start(out=txv[:, 6:8], in_=xv_r4[:, 6:8])
            # third tier: edge + wo (needed from head 0 / the output stage)
            nc.sync.dma_start(out=ed_pairs[0], in_=edge_r2[0])
            nc.sync.dma_start(out=ed_pairs[1], in_=edge_r2[1])
            nc.gpsimd.dma_start(out=ed_pairs[2], in_=edge_r2[2])
            nc.sync.dma_start(out=ed_pairs[3], in_=edge_r2[3])
            nc.gpsimd.dma_start(
                out=two, in_=wo_d.rearrange("p (c d) -> p c d", d=DO)
            )

            # PE clock-ramp filler on the memset tile (no DMA dependency)
            def junk(n, pool=None, name="jt"):
                jt = (pool or prjp).tile([P, STR], f32, tag="prj" if pool is None else "big", name=name)
                for _ in range(n):
                    nc.tensor.matmul(
                        jt[:], lhsT=tjk[:, 0:P], rhs=tjk[:], start=True, stop=True
                    )

            junk(NJUNK0)

            # ------- critical projections: q-ch0 s0 first (its xq chunks
            # land first), then k-ch0 s0; k-ch0 s1 (kcB) is deferred via
            # wait-until so its txk-s1 DMA wait can't block the first
            # scores in the in-order PE queue. -------
            ptqa = bigp.tile([P, STR], f32, tag="big")
            for cd in range(CD):
                nc.tensor.matmul(
                    ptqa[:],
                    lhsT=twq[:, cd, 0:P],
                    rhs=txq[:, 0, cd, :],
                    start=(cd == 0),
                    stop=(cd == CD - 1),
                )
            kcA = pvp.tile([P, STR], f32, tag="pv")
            for cd in range(CD):
                nc.tensor.matmul(
                    kcA[:],
                    lhsT=twk[:, cd, 0:P],
                    rhs=txk[:, 0, cd, :],
                    start=(cd == 0),
                    stop=(cd == CD - 1),
                )
            # biases for the first scores as soon as their psum lands
            nc.vector.tensor_scalar_add(
                out=tqh[:, 0, sl(0)], in0=ptqa[:], scalar1=tb4[:, 0, :]
            )
            nc.vector.tensor_scalar_add(
                out=khp[0:DK, 0, sl(0)], in0=kcA[0:DK, :], scalar1=tb4[0:DK, 2, :]
            )
            nc.vector.tensor_scalar_add(
                out=khp[DK:P, 1, sl(0)], in0=kcA[DK:P, :], scalar1=tb4[DK:P, 2, :]
            )
            kcB = pvp.tile([P, STR], f32, tag="pv")
            with tc.tile_wait_until(0.0166):
                for cd in range(CD):
                    nc.tensor.matmul(
                        kcB[:],
                        lhsT=twk[:, cd, 0:P],
                        rhs=txk[:, 1, cd, :],
                        start=(cd == 0),
                        stop=(cd == CD - 1),
                    )
                nc.vector.tensor_scalar_add(
                    out=khp[0:DK, 0, sl(1)], in0=kcB[0:DK, :], scalar1=tb4[0:DK, 2, :]
                )
                nc.vector.tensor_scalar_add(
                    out=khp[DK:P, 1, sl(1)], in0=kcB[DK:P, :], scalar1=tb4[DK:P, 2, :]
                )

            # ---------------- flat software-pipelined main loop -----------
            # steps: (head, kt) in head order 1, 0, 2, 3.
            HEADS = (1, 0, 2, 3)
            steps = [(h, kt) for h in HEADS for kt in range(KT)]

            # PE fillers threaded into head-1 steps (index within head 1).
            # vproj pair j covers k-tiles 2j, 2j+1 (8 mms each); ch1
            # projections (ptqb = q-ch1, ptk2 = k-ch1) ride the prjp ring.
            def vproj_pair(j):
                vp = prjp.tile([P, STR], f32, tag="prj")
                for u in range(2):
                    for cd in range(CD):
                        nc.tensor.matmul(
                            vp[:, u * DH : (u + 1) * DH],
                            lhsT=txv[:, 2 * j + u, cd, :],
                            rhs=twv[:, cd, :],
                            start=(cd == 0),
                            stop=(cd == CD - 1),
                        )
                # copy into the augmented-vh layout (DVE)
                nc.vector.tensor_copy(
                    out=tvha[:, 2 * j : 2 * j + 2, 0 : NHC * (DK + 1)].rearrange(
                        "p t (h w) -> p t h w", w=DK + 1
                    )[:, :, :, 0:DK],
                    in_=vp[:].rearrange("p (t h d) -> p t h d", t=2, h=NHC),
                )

            def qch1_stripe(s):
                pt = prjp.tile([P, STR], f32, tag="prj")
                for cd in range(CD):
                    nc.tensor.matmul(
                        pt[:],
                        lhsT=twq[:, cd, P : 2 * P],
                        rhs=txq[:, s, cd, :],
                        start=(cd == 0),
                        stop=(cd == CD - 1),
                    )
                nc.vector.tensor_scalar_add(
                    out=tqh[:, 1, sl(s)], in0=pt[:], scalar1=tb4[:, 1, :]
                )

            def kch1_stripe(s):
                pt = prjp.tile([P, STR], f32, tag="prj")
                for cd in range(CD):
                    nc.tensor.matmul(
                        pt[:],
                        lhsT=twk[:, cd, P : 2 * P],
                        rhs=txk[:, s, cd, :],
                        start=(cd == 0),
                        stop=(cd == CD - 1),
                    )
                nc.vector.tensor_scalar_add(
                    out=khp[0:DK, 2, sl(s)], in0=pt[0:DK, :], scalar1=tb4[0:DK, 3, :]
                )
                nc.vector.tensor_scalar_add(
                    out=khp[DK:P, 3, sl(s)], in0=pt[DK:P, :], scalar1=tb4[DK:P, 3, :]
                )

            # pre-accumulated ch0 output-projection for m-tiles 0/1 (their
            # prjp slots stay pinned until the tail adds ch1 on top)
            po_pre = {}

            def po_ch0(m):
                po = prjp.tile([P, DO], f32, tag="prj", name=f"pre{m}")
                nc.tensor.matmul(
                    po[:],
                    lhsT=tot[:, 0, m * P : (m + 1) * P],
                    rhs=two[:, 0, :],
                    start=True,
                    stop=False,
                    skip_group_check=True,
                )
                po_pre[m] = po

            def qch0_s1():
                pt = prjp.tile([P, STR], f32, tag="prj")
                for cd in range(CD):
                    nc.tensor.matmul(
                        pt[:],
                        lhsT=twq[:, cd, 0:P],
                        rhs=txq[:, 1, cd, :],
                        start=(cd == 0),
                        stop=(cd == CD - 1),
                    )
                nc.vector.tensor_scalar_add(
                    out=tqh[:, 0, sl(1)], in0=pt[:], scalar1=tb4[:, 0, :]
                )

            fillers = {
                (0, 0): lambda: kch1_stripe(1),
                (3, 3): lambda: po_ch0(0),
                (3, 4): lambda: po_ch0(1),
            }
            fillers_A = {
                2: lambda: vproj_pair(0),
                3: lambda: (qch0_s1(), vproj_pair(1)),
            }
            fillers_B = {
                0: lambda: vproj_pair(2),
                1: lambda: vproj_pair(3),
                2: lambda: qch1_stripe(0),
                3: lambda: (qch1_stripe(1), kch1_stripe(0)),
            }

            # deferred normalize for the previous head, hooked into the next
            # head's step 1 (PV of kt7 lands there too); the reciprocal
            # reads the denominator row straight out of PSUM
            def norm_stripe(h, pvs, s):
                rr = rrp.tile([1, STR], f32, tag="rr")
                rs = rrp.tile([1, STR], f32, tag="rs")
                nc.vector.tensor_copy(out=rs[:], in_=pvs[s][DK : DK + 1, :])
                nc.vector.reciprocal_approx_fast(out=rr[:], in_=rs[:])
                rb = rbp.tile([DK, STR], f32, tag="rb")
                nc.gpsimd.partition_broadcast(rb[:], rr[:])
                ch, off = h // 2, (h % 2) * DK
                nc.vector.tensor_mul(
                    tot[off : off + DK, ch, sl(s)], pvs[s][0:DK, :], rb[:]
                )

            # main loop state
            pv_by_head = {}
            te_by_step = {}
            te_h1 = {}
            prev_head = {1: None, 0: 1, 2: 0, 3: 2}

            def emit_pv(h, kt, stop):
                pvs = pv_by_head[h]
                te = te_by_step[(h, kt)]
                for s in range(NS):
                    nc.tensor.matmul(
                        pvs[s][:],
                        lhsT=tvha[:, kt, h * (DK + 1) : h * (DK + 1) + P],
                        rhs=te[:, sl(s)],
                        start=(kt == 0),
                        stop=stop,
                    )

            def pv1_mm(kt, s, stop):
                te, u = te_h1[(kt, s)]
                nc.tensor.matmul(
                    pv_by_head[1][s][:],
                    lhsT=tvha[:, kt, 1 * (DK + 1) : 1 * (DK + 1) + P],
                    rhs=te[:, sl(u)],
                    start=(kt == 0),
                    stop=stop,
                )

            def pv1_pair(p, s, stop_last):
                pv1_mm(2 * p, s, False)
                pv1_mm(2 * p + 1, s, stop_last)

            # Logical clock: wait-until hints make the Tile scheduler's sim
            # order instructions the way the real hardware needs them —
            # scores/exp first within a step, then fillers/hooks, then the
            # lagged PVs.  (The sim's fast DMA model otherwise hoists filler
            # matmuls ahead of critical scores; the in-order PE queue then
            # stalls on late DMAs.)
            #
            # Head 1 runs as 8 pair-steps: stripe 0 of k-tile pairs 0..3
            # (phase A, needs only xq stripe 0), then stripe 1 (phase B) —
            # xq stripe 1 thus leaves the critical DMA path.  Each pair-step
            # exps one [P, SEQ] tile holding two k-tiles' half-scores, so
            # the Act engine efficiency is unchanged.
            TA, TSTEP = 15.0, 1.2

            pv_by_head[1] = (
                pvp.tile([P, STR], f32, tag="pv", name="pv1s0"),
                pvp.tile([P, STR], f32, tag="pv", name="pv1s1"),
            )
            for p in range(8):
                s, j = p // 4, p % 4
                base = TA + TSTEP * p
                with tc.tile_wait_until(base / 1000.0):
                    stt = bigp.tile([P, SEQ], f32, tag="big")
                    for u in (0, 1):
                        kt = 2 * j + u
                        nc.tensor.matmul(
                            stt[:, sl(u)],
                            lhsT=khp[:, 1, kt * P : (kt + 1) * P],
                            rhs=tqh[:, 0, sl(s)],
                            start=True,
                            stop=True,
                        )
                    te = expp.tile([P, SEQ], cdt, tag="expT")
                    nc.scalar.activation(out=te, in_=stt[:], func=Exp)
                    for u in (0, 1):
                        te_h1[(2 * j + u, s)] = (te, u)
                f = (fillers_A if s == 0 else fillers_B).get(j)
                if f is not None:
                    with tc.tile_wait_until((base + 0.4) / 1000.0):
                        f()
                if p >= 3:
                    # lag-3 over the pair-step sequence (vproj pairs chase
                    # the xv DMA stream one step ahead of their PVs)
                    pp = p - 3
                    ps, pj = pp // 4, pp % 4
                    with tc.tile_wait_until((base + 0.8) / 1000.0):
                        pv1_pair(pj, ps, stop_last=(pj == 3))

            # heads 0, 2, 3 as full steps
            T0 = TA + TSTEP * 8
            steps = [(h, kt) for h in (0, 2, 3) for kt in range(KT)]
            for i, (h, kt) in enumerate(steps):
                ch = h // 2
                base = T0 + TSTEP * i
                # allocate this head's pv tiles at its first step
                if kt == 0:
                    pv_by_head[h] = (
                        pvp.tile([P, STR], f32, tag="pv", name=f"pv{h}s0"),
                        pvp.tile([P, STR], f32, tag="pv", name=f"pv{h}s1"),
                    )
                with tc.tile_wait_until(base / 1000.0):
                    # scores for (h, kt); head 0 accumulates I @ edgeT on top
                    stt = bigp.tile([P, SEQ], f32, tag="big")
                    for s in range(NS):
                        nc.tensor.matmul(
                            stt[:, sl(s)],
                            lhsT=khp[:, h, kt * P : (kt + 1) * P],
                            rhs=tqh[:, ch, sl(s)],
                            start=True,
                            stop=(h != 0),
                        )
                        if h == 0:
                            nc.tensor.matmul(
                                stt[:, sl(s)],
                                lhsT=teye[:],
                                rhs=ed_pairs[kt // 2][:, kt % 2, sl(s)],
                                start=False,
                                stop=True,
                            )
                    # exp on the Act engine
                    te = expp.tile([P, SEQ], cdt, tag="expT")
                    nc.scalar.activation(out=te, in_=stt[:], func=Exp)
                    te_by_step[(h, kt)] = te
                # PE fillers for this step
                f = fillers.get((h, kt))
                if f is not None:
                    with tc.tile_wait_until((base + 0.4) / 1000.0):
                        f()
                # lagged PV matmuls; head 1's stripe-1 tail PVs land in
                # head 0's first two steps, followed by the normalize hooks
                # that free the pvp ring
                ph = prev_head[h]
                if h == 0 and kt == 0:
                    with tc.tile_wait_until((base + 0.8) / 1000.0):
                        pv1_pair(1, 1, stop_last=False)
                elif h == 0 and kt == 1:
                    with tc.tile_wait_until((base + 0.5) / 1000.0):
                        pv1_pair(2, 1, stop_last=False)
                        pv1_pair(3, 1, stop_last=True)
                        norm_stripe(1, pv_by_head[1], 0)
                        norm_stripe(1, pv_by_head[1], 1)
                elif kt == 0 and ph is not None:
                    with tc.tile_wait_until((base + 0.8) / 1000.0):
                        emit_pv(ph, KT - 2, stop=False)
                elif kt == 1 and ph is not None:
                    with tc.tile_wait_until((base + 0.5) / 1000.0):
                        emit_pv(ph, KT - 1, stop=True)
                        norm_stripe(ph, pv_by_head[ph], 0)
                        norm_stripe(ph, pv_by_head[ph], 1)
                elif kt >= 2:
                    with tc.tile_wait_until((base + 0.8) / 1000.0):
                        emit_pv(h, kt - 2, stop=False)

            # ---------------- tail ----------------
            TT = T0 + TSTEP * len(steps)
            h_last = HEADS[-1]
            with tc.tile_wait_until(TT / 1000.0):
                emit_pv(h_last, KT - 2, stop=False)
                junk(2, pool=bigp, name="jtt0")
                emit_pv(h_last, KT - 1, stop=True)

            # h3 normalize, per stripe; stripe 0 gates out-proj m 0-3.
            # denominator copy on the Act engine (idle after the last exp)
            pvs3 = pv_by_head[h_last]
            ch3, off3 = h_last // 2, (h_last % 2) * DK

            def norm_tail(s):
                rs = rrp.tile([1, STR], f32, tag="rs")
                if s == 0:
                    nc.scalar.copy(out=rs[:], in_=pvs3[s][DK : DK + 1, :])
                else:
                    nc.vector.tensor_copy(out=rs[:], in_=pvs3[s][DK : DK + 1, :])
                rr = rrp.tile([1, STR], f32, tag="rr")
                nc.vector.reciprocal_approx_fast(out=rr[:], in_=rs[:])
                rb = rbp.tile([DK, STR], f32, tag="rb")
                nc.gpsimd.partition_broadcast(rb[:], rr[:])
                nc.vector.tensor_mul(
                    tot[off3 : off3 + DK, ch3, sl(s)], pvs3[s][0:DK, :], rb[:]
                )

            with tc.tile_wait_until((TT + 0.6) / 1000.0):
                norm_tail(0)
                junk(9, pool=bigp, name="jtt1")
                norm_tail(1)

            # output projection, one po mm per chunk per m-tile, spread over
            # six 1-bank psum slots (2 pinned pre-tiles + pvp + bigp ring);
            # casts alternate DVE/Act, stores alternate the sync/gpsimd
            # queues (the scalar engine stays cast-only)
            cast_ops = [
                lambda o, i: nc.vector.tensor_copy(out=o, in_=i),
                lambda o, i: nc.scalar.copy(out=o, in_=i),
            ]
            store_eng = [nc.sync, nc.gpsimd, nc.scalar]

            def po_tail(m, po, ch0_done):
                if not ch0_done:
                    nc.tensor.matmul(
                        po[:],
                        lhsT=tot[:, 0, m * P : (m + 1) * P],
                        rhs=two[:, 0, :],
                        start=True,
                        stop=False,
                        skip_group_check=True,
                    )
                nc.tensor.matmul(
                    po[:],
                    lhsT=tot[:, 1, m * P : (m + 1) * P],
                    rhs=two[:, 1, :],
                    start=False,
                    stop=True,
                    skip_group_check=True,
                )
                oal = oalp.tile([P, DO], cdt, tag="oall")
                cast_ops[m % 2](oal[:], po[:])
                store_eng[m % 3].dma_start(out=out_r[:, m], in_=oal[:])

            with tc.tile_wait_until((TT + 1.4) / 1000.0):
                po_tail(0, po_pre[0], True)
                po_tail(1, po_pre[1], True)
            with tc.tile_wait_until((TT + 2.0) / 1000.0):
                for m in (2, 3):
                    po = pvp.tile([P, DO], f32, tag="pv", name=f"po{m}")
                    po_tail(m, po, False)
            with tc.tile_wait_until((TT + 2.6) / 1000.0):
                for m in (4, 5):
                    po = bigp.tile([P, DO], f32, tag="big", name=f"po{m}")
                    po_tail(m, po, False)
            with tc.tile_wait_until((TT + 3.2) / 1000.0):
                for m in (6, 7):
                    po = prjp.tile([P, DO], f32, tag="prj", name=f"po{m}")
                    po_tail(m, po, False)

    nc.compile()
    _nc = nc
    return nc


def _in_maps(q, k, v, edge_matrix, Wq, bq, Wk, bk, Wv, Wo):
    dt = _np_dt()
    zeros_edge = np.zeros((SEQ, SEQ), dt)
    edge_t = np.ascontiguousarray(edge_matrix.T).astype(dt)

    def re_cp(m):
        # [C*P, D] -> [P, C*D] (partition-major packing of "(c p) d -> p c d")
        cp, d = m.shape
        return np.ascontiguousarray(
            m.reshape(cp // P, P, d).transpose(1, 0, 2).reshape(P, -1)
        )

    def re_sp(m):
        # [CD*P, NS*STR] -> [P, NS*CD*STR] stripe-major packing
        return np.ascontiguousarray(
            m.reshape(CD, P, NS, STR).transpose(1, 2, 0, 3).reshape(P, -1)
        )

    def re_st(m):
        # [CD*P, KT*P] -> [P, KT*CD*P]: kt-major packing for the v stream
        return np.ascontiguousarray(
            m.reshape(CD, P, KT, P).transpose(1, 2, 0, 3).reshape(P, -1)
        )

    xt = {}
    for b in range(B):
        xt[b] = (
            re_sp(np.ascontiguousarray(q[b].T).astype(dt)),
            re_sp(np.ascontiguousarray(k[b].T).astype(dt)),
            re_st(np.ascontiguousarray(v[b].T).astype(dt)),
        )
    maps = []
    for c in range(8):
        b, g = c // 2, c % 2
        is_edge = g == 0 and b < 2
        rows = slice(g * DH, (g + 1) * DH)
        wq_c = np.ascontiguousarray(Wq[rows].T) * np.float32(1.0 / 8.0)
        bq_c = (bq[rows] * np.float32(1.0 / 8.0)).copy()
        if is_edge:
            wq_c[:, 0:DK] = 0.0
            bq_c[0:DK] = 0.0
        maps.append(
            {
                "eye": np.eye(P, dtype=dt),
                "wq": re_cp(wq_c.astype(dt)),
                "wk": re_cp(np.ascontiguousarray(Wk[rows].T).astype(dt)),
                "wv": re_cp(np.ascontiguousarray(Wv[rows].T).astype(dt)),
                "wo": re_cp(np.ascontiguousarray(Wo[:, rows].T).astype(dt)),
                "xq": xt[b][0],
                "xk": xt[b][1],
                "xv": xt[b][2],
                "bqk": np.concatenate([bq_c, bk[rows]]).reshape(2 * DH, 1),
                "edge": edge_t if is_edge else zeros_edge,
            }
        )
    return maps


def _ensure_ntff_hook():
    """Register the axon NTFF profile hook if the image's antenv lacks it."""
    import contextlib
    import ctypes
    import types

    try:
        from antenv.axon_hooks import get_axon_ntff_profile_hook  # noqa: F401
        return
    except ImportError:
        pass

    so_path = "/opt/axon/libaxon_pjrt.so"
    try:
        lib = ctypes.CDLL(so_path)
    except OSError:
        return
    if not hasattr(lib, "axon_start_nrt_profile"):
        return
    lib.axon_start_nrt_profile.argtypes = [
        ctypes.POINTER(ctypes.c_int64),
        ctypes.c_size_t,
    ]
    lib.axon_start_nrt_profile.restype = ctypes.c_int64
    lib.axon_stop_nrt_profile.argtypes = [ctypes.c_char_p]
    lib.axon_stop_nrt_profile.restype = ctypes.c_int64

    @contextlib.contextmanager
    def _hook(output_dir, device_ids):
        import jax

        jax.devices()
        if device_ids:
            ids = (ctypes.c_int64 * len(device_ids))(*device_ids)
            rc = lib.axon_start_nrt_profile(ids, len(device_ids))
        else:
            rc = lib.axon_start_nrt_profile(None, 0)
        if rc != 0:
            raise RuntimeError(f"axon_start_nrt_profile rc={rc}")
        try:
            yield
        finally:
            n = lib.axon_stop_nrt_profile(str(output_dir).encode())
            if n < 0:
                raise RuntimeError(f"axon_stop_nrt_profile rc={n}")

    _state = {"hook": _hook}
    mod = types.ModuleType("antenv.axon_hooks")
    mod.get_axon_ntff_profile_hook = lambda: _state["hook"]
    mod.set_axon_ntff_profile_hook = lambda h: _state.__setitem__("hook", h)
    import antenv

    antenv.axon_hooks = mod
    sys.modules["antenv.axon_hooks"] = mod


def kernel(q, k, v, edge_matrix, Wq, bq, Wk, bk, Wv, bv, Wo, bo, _trace=False):
    from concourse.bass_utils import run_bass_kernel_spmd

    if _trace:
        _ensure_ntff_hook()

    q, k, v = (np.asarray(t, np.float32) for t in (q, k, v))
    edge_matrix = np.asarray(edge_matrix, np.float32)
    Wq, bq, Wk, bk, Wv, bv, Wo, bo = (
        np.asarray(t, np.float32) for t in (Wq, bq, Wk, bk, Wv, bv, Wo, bo)
    )

    nc = _build()
    maps = _in_maps(q, k, v, edge_matrix, Wq, bq, Wk, bk, Wv, Wo)
    res = run_bass_kernel_spmd(nc, maps, core_ids=list(range(8)), trace=_trace)

    bo_eff = bo + Wo @ bv
    out = np.empty((B, SEQ, DO), np.float32)
    for b in range(B):
        out[b] = res.results[2 * b]["outp"] + res.results[2 * b + 1]["outp"] + bo_eff
    if _trace:
        return out, res
    return out
